# revision 9
# baseline (speedup 1.0000x reference)
"""Trainium2 Bass kernel for BF16IndexerBaseline (sparse_attention).

Computes, for q:(1,M,H,D) bf16, k:(1,N,D) bf16, weights:(H,M) bf16:

    index_score[b,m,n] = sum_h relu(q[b,m,h,:] . k[b,n,:]) * (weights[h,m]*D**-0.5)

Strategy (8 NeuronCores, SPMD, host-side sharding of m):
  - each core gets an m-shard of 256 rows (2 m-tiles of 128), k replicated.
  - weights >= 0, so the per-(m,h) scale commutes with relu and is folded
    into q ON THE HOST: q' = q * (w*scale) -> the device computes plain
    sum_h relu(q'.k). Both q' and k are pre-transposed on the host into
    [D=128, cols] layout, so the kernel has NO device-side transposes, no
    weights load, and no per-partition scale vectors.
  - per (m-tile, n-chunk of 1024) unit: 16 heads x 2 matmuls (K=D=128,
    stationary qT block, moving kT) -> fp32 logits in PSUM [128,1024]
    tiles (2-buf psA pool for ACT-heads, 2-buf psV for DVE-heads).
  - eviction split (PSUM reads: 1 elem/lane/cyc fp32, ACT+DVE only):
      * 9 "A" heads on ScalarE: relu -> bf16 leaves (~1.20us/tile).
      * 7 "V" heads on VectorE via the fused custom DVE op
        RELU_SCALE_ADD: acc = relu(psum) + in1 (~1.28us/tile, the
        accumulate is free). c0 seeds from leaf r0 (evicted 2 slots
        earlier); the last chain op folds acc and writes the bf16 stage.
      * leaf tree kept OFF VectorE: 4 in-place SWDGE ring adds + 2
        GpSimd adds per unit; only the final two bf16 merges (x=r1+t,
        stage+=x, ~0.68us each) run on VectorE, in the next unit's slack.
  - output bf16 (halves out-DMA traffic; host casts to fp32).
  - last unit: no ring/GpSimd ops on the critical tail; trailing heads
    split across both engines and the tree finishes on VectorE.
  - PE warm-up burst at t=0 trips the HAM clock gate to 2.4 GHz; a dummy
    ACTIVATE preloads the relu table set.
"""

import os

os.environ.setdefault("MYCRO_LOCAL_CACHE", "1")

import numpy as np
import ml_dtypes
from contextlib import ExitStack

import concourse.bass as bass
import concourse.tile as tile
from concourse import bacc, mybir
from concourse.bass_utils import run_bass_kernel_spmd

# ---------------------------------------------------------------- problem dims
B = 1
M = 2048
H = 16
N = 4096
D = 128
N_CORES = 8
MS = M // N_CORES          # 256 rows of m per core
MT = MS // 128             # 2 m-tiles per core
FD = 1024                  # n-chunk (free dim) per eviction op = 2 PSUM banks
NCH = N // FD              # 4 n-chunks
WARMUP_MMS = int(os.environ.get("IDX_WARMUP_MMS", "5"))
MM_W = 512                 # matmul moving width (1 PSUM bank fp32)

# steady-unit role string: A = ScalarE relu leaf, V = VectorE chain head.
ROLES = os.environ.get("IDX_ROLES", "AVAVAVAVAVAVAVAA")
ROLES_LAST = os.environ.get("IDX_ROLES_LAST", "AVAVAVAVAVAVAVAD")
# D = DVE TS eviction (leaf evicted on VectorE; used near the tail)

BF16 = mybir.dt.bfloat16
F32 = mybir.dt.float32
SCALE_BF16 = float(np.float32(np.array(D ** -0.5, dtype=ml_dtypes.bfloat16)))

# --------------------------------------------------- custom fused DVE op
# out = relu(in0 * s0) + in1   (s0 per-partition scalar [P,1]; used with ones)
import concourse.dve_ops as dve_ops
from concourse.dve_spec import Spec as _Spec, Src0 as _Src0, Src1 as _Src1, C0 as _C0
from concourse.dve_spec import relu as _relu, lower as _lower
from concourse.dve_uop import DveOpSpec as _DveOpSpec

_OP_NAME = "RELU_SCALE_ADD_ANT"


def _ref_relu_scale_add(in0, in1, s0, s1, imm2):
    x = np.nan_to_num(in0.astype(np.float32) * s0, nan=0.0, posinf=np.inf, neginf=-np.inf)
    return np.maximum(x, 0.0).astype(np.float32) + in1


def _register_relu_scale_add():
    for op in dve_ops.OPS:
        if op.name == _OP_NAME:
            return op
    spec = _Spec(body=_relu(_Src0 * _C0) + _Src1, reference=_ref_relu_scale_add)
    row = max(dve_ops._SUB_OPCODE_FOR_NAME.values()) + 1
    assert row < 0x20
    dve_ops._SUB_OPCODE_FOR_NAME[_OP_NAME] = row
    shas = {
        v: _DveOpSpec(name=_OP_NAME, opcode=row, uops=_lower(spec, ver=v), rd1_en=True).sha(v)
        for v in ("v3", "v4")
    }
    op = dve_ops.DveOp(_OP_NAME, spec, subdim=False, uops_sha=shas)
    dve_ops.OPS.append(op)
    dve_ops.CUSTOM_DVE_SPECS[_OP_NAME] = spec
    return op


RELU_SCALE_ADD = _register_relu_scale_add()


# ------------------------------------------------------------------ kernel IR
def _emit(ctx: ExitStack, tc: "tile.TileContext", q_d, k_d, o_d):
    nc = tc.nc
    AOp = mybir.AluOpType

    const = ctx.enter_context(tc.tile_pool(name="const", bufs=1))
    psA = ctx.enter_context(tc.tile_pool(name="psA", bufs=2, space="PSUM"))
    psV = ctx.enter_context(tc.tile_pool(name="psV", bufs=2, space="PSUM"))
    rpool = ctx.enter_context(tc.tile_pool(name="rpool", bufs=24))
    tpool = ctx.enter_context(tc.tile_pool(name="tpool", bufs=8))
    apool = ctx.enter_context(tc.tile_pool(name="apool", bufs=3))
    opool = ctx.enter_context(tc.tile_pool(name="opool", bufs=4))

    # ---- t=0 dummies: bf16 zero tile (VectorE memset) feeds a warm-up MM
    # burst (HAM -> 2.4 GHz); a 1-col ACTIVATE preloads the relu table set.
    dummy = const.tile([128, 512], BF16)
    nc.vector.memset(dummy[:], 0.0)
    if WARMUP_MMS:
        wu_ps = psA.tile([128, FD], F32, tag="logits", name="wu_ps")
        for i in range(WARMUP_MMS):
            nc.tensor.matmul(
                wu_ps[:, 0:512], dummy[:, 0:128], dummy[:], start=True, stop=True
            )
    # ---- input loads (plain 2D DMA, no transposes): pieces ordered so unit
    # 0's data lands first, alternating across the sync + scalar HWDGE rings.
    # The relu-table preload (d_act) sits after the first kT piece so the
    # ~2.7us ACT_TABLE_LOAD overlaps the remaining loads.
    kT = const.tile([128, N], BF16)
    qT = const.tile([128, H * MS], BF16)          # col = mt*H*128 + h*128 + m
    QP = H * 128                                  # one m-tile's worth of q cols
    nc.scalar.dma_start(out=kT[:, 0:FD], in_=k_d[:, 0:FD])
    nc.sync.dma_start(out=qT[:, 0:512], in_=q_d[:, 0:512])
    d_act = const.tile([128, 1], BF16)
    nc.scalar.activation(d_act[:], dummy[:, 0:1], mybir.ActivationFunctionType.Relu)
    nc.sync.dma_start(out=qT[:, 512:QP], in_=q_d[:, 512:QP])
    nc.scalar.dma_start(out=kT[:, FD:2 * FD], in_=k_d[:, FD:2 * FD])
    nc.sync.dma_start(out=kT[:, 2 * FD:N], in_=k_d[:, 2 * FD:N])
    nc.sync.dma_start(out=qT[:, QP:2 * QP], in_=q_d[:, QP:2 * QP])

    # ones vector for the custom op's per-partition scale
    ones = const.tile([128, 1], F32)
    nc.vector.memset(ones[:], 1.0)

    n_units = MT * NCH
    uidx = 0
    pending = None      # deferred finisher of the previous unit
    for mt in range(MT):
        for nci in range(NCH):
            n0 = nci * FD
            uid = f"{mt}_{nci}"
            last = uidx == n_units - 1
            roles = ROLES_LAST if last else ROLES
            uidx += 1

            acc = apool.tile([128, FD], F32, tag="acc", name=f"acc_{uid}")
            stage = opool.tile([128, FD], BF16, tag="stage", name=f"stage_{uid}")
            leaves = []         # bf16 leaf tiles in eviction order
            n_v = roles.count("V")
            vi = 0              # chain index

            def _mk_head(h, pool, uid=uid, n0=n0):
                pt = pool.tile([128, FD], F32, tag="logits", name=f"ps_{uid}_{h}")
                lhs = qT[:, mt * QP + h * 128: mt * QP + h * 128 + 128]
                for j in range(FD // MM_W):
                    nc.tensor.matmul(
                        pt[:, j * MM_W: (j + 1) * MM_W],
                        lhs,
                        kT[:, n0 + j * MM_W: n0 + (j + 1) * MM_W],
                        start=True,
                        stop=True,
                    )
                return pt

            for h, role in enumerate(roles):
                if h == 9 and pending is not None:
                    # previous unit's finisher: its GpSimd-built partial (t)
                    # is ready by now, so these two bf16 VectorE adds slot
                    # in without stalling this unit's chain stream.
                    pending()
                    pending = None
                if role == "A" or role == "D":
                    pt = _mk_head(h, psA if role == "A" else psV)
                    r = rpool.tile([128, FD], BF16, tag="r", name=f"r_{uid}_{h}")
                    if role == "A":
                        nc.scalar.activation(
                            r[:], pt[:], mybir.ActivationFunctionType.Relu
                        )
                    else:
                        nc.vector.tensor_scalar(
                            r[:], pt[:], 1.0, 0.0, op0=AOp.mult, op1=AOp.max
                        )
                    leaves.append(r)
                    li = len(leaves) - 1
                    if not last:
                        # steady-unit tree, fully decoupled from the chain:
                        # rings fold r0..r5 into r0 (pairs emitted only once
                        # their inputs' evictions are in flight, merges only
                        # after the feeding ring adds have completed, so the
                        # GpSimd queue never blocks); GpSimd sums r6..r8
                        # into t; VectorE finishes x = r0 + t in next-unit
                        # slack (the deferred finisher).
                        if li == 1:
                            nc.gpsimd.dma_start(
                                out=leaves[0][:], in_=leaves[1][:],
                                accum_op=AOp.add,
                            )
                        elif li == 3:
                            nc.gpsimd.dma_start(
                                out=leaves[2][:], in_=leaves[3][:],
                                accum_op=AOp.add,
                            )
                        elif li == 5:
                            nc.gpsimd.dma_start(
                                out=leaves[0][:], in_=leaves[2][:],
                                accum_op=AOp.add,
                            )
                            nc.gpsimd.dma_start(
                                out=leaves[4][:], in_=leaves[5][:],
                                accum_op=AOp.add,
                            )
                        elif li == 7:
                            nc.gpsimd.dma_start(
                                out=leaves[0][:], in_=leaves[4][:],
                                accum_op=AOp.add,
                            )
                        elif li == 8:
                            t = tpool.tile([128, FD], BF16, tag="t", name=f"t_{uid}")
                            nc.gpsimd.tensor_add(t[:], leaves[6][:], leaves[7][:])
                            nc.gpsimd.tensor_add(t[:], t[:], leaves[8][:])
                else:  # V chain head
                    pt = _mk_head(h, psV)
                    if vi == 0:
                        nc.vector.tensor_scalar(
                            acc[:], pt[:], 1.0, 0.0, op0=AOp.mult, op1=AOp.max
                        )
                    elif vi == n_v - 1:
                        nc.vector._custom_dve(
                            RELU_SCALE_ADD, out=stage[:], in0=pt[:],
                            in1=acc[:], s0=ones[:, 0:1],
                        )
                    else:
                        nc.vector._custom_dve(
                            RELU_SCALE_ADD, out=acc[:], in0=pt[:],
                            in1=acc[:], s0=ones[:, 0:1],
                        )
                    vi += 1

            if last:
                if pending is not None:
                    pending()
                    pending = None
                # tail tree entirely on VectorE/GpSimd: GpSimd handles the
                # early-mid leaves with plenty of slack; VectorE folds the
                # late ones right after their evictions.
                t1 = tpool.tile([128, FD], BF16, tag="t", name=f"t_{uid}")
                nc.gpsimd.tensor_add(t1[:], leaves[0][:], leaves[1][:])
                nc.gpsimd.tensor_add(t1[:], t1[:], leaves[2][:])
                nc.gpsimd.tensor_add(t1[:], t1[:], leaves[3][:])
                x = tpool.tile([128, FD], BF16, tag="x", name=f"x_{uid}")
                nc.vector.tensor_add(x[:], leaves[4][:], leaves[5][:])
                nc.vector.tensor_add(x[:], x[:], t1[:])
                nc.vector.tensor_add(x[:], x[:], leaves[6][:])
                nc.vector.tensor_add(x[:], x[:], leaves[7][:])
                nc.vector.tensor_add(x[:], x[:], leaves[8][:])
                nc.vector.tensor_add(stage[:], stage[:], x[:])
                nc.sync.dma_start(
                    out=o_d[mt * 128: (mt + 1) * 128, n0: n0 + FD],
                    in_=stage[:],
                )
            else:
                def _finish(leaves=leaves, t=t, stage=stage, t_uid=uid,
                            mt=mt, n0=n0):
                    # x = (r0..r5) + (r6..r8); out = stage + x
                    x = tpool.tile([128, FD], BF16, tag="x", name=f"x_{t_uid}")
                    nc.vector.tensor_add(x[:], leaves[0][:], t[:])
                    nc.vector.tensor_add(stage[:], stage[:], x[:])
                    nc.sync.dma_start(
                        out=o_d[mt * 128: (mt + 1) * 128, n0: n0 + FD],
                        in_=stage[:],
                    )
                pending = _finish


_NC_CACHE = None


def _build():
    global _NC_CACHE
    if _NC_CACHE is not None:
        return _NC_CACHE
    nc = bacc.Bacc(
        "TRN2",
        target_bir_lowering=False,
        debug=False,
        enable_asserts=False,
        num_devices=N_CORES,
    )
    q_d = nc.dram_tensor("qT", [D, H * MS], BF16, kind="ExternalInput").ap()
    k_d = nc.dram_tensor("kT", [D, N], BF16, kind="ExternalInput").ap()
    o_d = nc.dram_tensor("o", [MS, N], BF16, kind="ExternalOutput").ap()
    with tile.TileContext(nc) as tc:
        with ExitStack() as ctx:
            _emit(ctx, tc, q_d, k_d, o_d)
    nc.compile()
    _NC_CACHE = (nc, q_d, k_d, o_d)
    return _NC_CACHE


def _shard_inputs(q, k, weights):
    bf16 = ml_dtypes.bfloat16
    q = np.asarray(q).astype(bf16, copy=False).reshape(M, H, D)
    k = np.asarray(k).astype(bf16, copy=False).reshape(N, D)
    w = np.asarray(weights).astype(bf16, copy=False).reshape(H, M)
    # q_s matches the reference's bf16 rounding: bf16(w) * bf16(scale) -> bf16
    q_s = (w.astype(np.float32) * np.float32(SCALE_BF16)).astype(bf16)
    # pre-scale q (weights >= 0 so the scale commutes with relu)
    q_scaled = (q.astype(np.float32) * q_s.T[:, :, None].astype(np.float32)).astype(bf16)
    kT = np.ascontiguousarray(k.T)                      # [D, N]
    in_maps = []
    for c in range(N_CORES):
        m0 = c * MS
        # cols ordered m-tile-major: col = mt*H*128 + h*128 + m_local
        q_c = q_scaled[m0: m0 + MS].reshape(MT, 128, H, D).transpose(0, 2, 1, 3)
        qT_c = np.ascontiguousarray(
            q_c.reshape(MT * H * 128, D).T                # [D, MT*H*128]
        )
        in_maps.append({"qT": qT_c, "kT": kT})
    return in_maps


LAST_RESULTS = None


def kernel(q, k, weights):
    global LAST_RESULTS
    nc, *_ = _build()
    in_maps = _shard_inputs(q, k, weights)
    trace = bool(int(os.environ.get("IDX_TRACE", "0")))
    res = run_bass_kernel_spmd(
        nc, in_maps, core_ids=list(range(N_CORES)), trace=trace
    )
    LAST_RESULTS = res
    out = np.empty((B, M, N), np.float32)
    for c in range(N_CORES):
        out[0, c * MS: (c + 1) * MS] = res.results[c]["o"].astype(np.float32)
    return out
